# revision 25
# baseline (speedup 1.0000x reference)
"""Causal multi-head attention on 8 Trainium2 NeuronCores.

Full module: x:(2,2048,1024) f32, 16 heads, head_dim 64, causal softmax
(scaled by 1/sqrt(1024)), out = attn(x) @ Wo + bo.

Sharding: core c handles batch b = c // 4 and head group g = c % 4
(4 heads = 2 head pairs, i.e. 256 columns of Wq/Wk/Wv and 256 rows of
Wo). Every core runs the same program (SPMD); the host sums the 4
per-group partial output projections per batch and adds the bias.

Per-core layout (all matmuls contract over the SBUF partition dim):
  xT   [f=1024, t=2048] bf16, transposed on the host, straight DMA in
  QT/KT[d, t] per head pair: partitions = d within pair (head A 0-63,
       head B 64-127), built from Wq/Wk as lhsT against xT
  V    [t, d] token-major [128, 16 chunks, 4 heads, 64+1] with a fused
       ones column per head (softmax sums fall out of the PV matmul)
  S^T  [k=128, q<=512] per chunk = KT-slice lhsT x QT rhs, two heads on
       disjoint 64-partition halves, diagonal chunks narrowed to the
       causal column range
  P^T  = exp(S^T/32) on ScalarE; diagonal 128-block masked by a bf16
       lower-triangle multiply on DVE
  ctx  [q=128, 65] per query sub-chunk = pt-slice lhsT x V rhs,
       accumulated over k chunks in PSUM; col 64 = softmax sums
  norm: reciprocal(sums) -> per-partition tensor_scalar multiply (DVE)
  ctxT via PE transpose of the normalized [q, d] tile
  out  [t, 1024] = ctxT lhsT x Wo rhs (partial over heads; host sums)

Emission is software-pipelined: the 80 S/exp chunk units pace the body
and every other PE-side unit (projections of later token blocks, PV,
transposes, output projection) is interleaved between chunks with a
small lag so no engine queue head-blocks on ScalarE.
"""

import os

import numpy as np

N = 2048        # tokens per batch
D = 1024        # model dim
HG = 4          # heads per core
HD = 64         # head dim
DG = HG * HD    # 256, feature columns per core
SCALE = 1.0 / 32.0  # 1/sqrt(D); module scales by sqrt(d_out), not head_dim
NCORES = 8

PT_BUFS = int(os.environ.get("PT_BUFS", "22"))
# gpsimd cannot read PSUM (BIR verifier) — drains stay on DVE
GPSIMD_DRAIN = os.environ.get("GPSIMD_DRAIN", "0") != "0"
# fp8 (e4m3) Q/K projections via DoubleRow matmuls (2x PE throughput).
# Wq/Wk are pre-scaled by W8SCALE into fp8 range; the exp() scale divides
# the resulting alpha^2 factor back out of the scores.
FP8_QK = os.environ.get("FP8_QK", "0") != "0"
W8SCALE = 64.0
# FENCE=1 serializes repetition bodies (for latency measurement): the next
# body's first x DMA is made to wait on the previous body's last output.
FENCE = os.environ.get("FENCE", "0") != "0"

_CACHE = {}


def _build_nc(repeat=1):
    from contextlib import ExitStack

    import concourse.mybir as mybir
    import concourse.tile as tile
    from concourse import bacc

    FP32 = mybir.dt.float32
    BF16 = mybir.dt.bfloat16
    FP8 = mybir.dt.float8e4
    EXP = mybir.ActivationFunctionType.Exp
    DR = mybir.MatmulPerfMode.DoubleRow

    NT = N // 128   # 16 token chunks
    NF = D // 128   # 8 feature chunks
    NQ = N // 512   # 4 query blocks

    nc = bacc.Bacc("TRN2", target_bir_lowering=False, debug=False)

    x_d = nc.dram_tensor("x", [D, N], BF16, kind="ExternalInput").ap()
    if FP8_QK:
        x8_d = nc.dram_tensor("x8", [D, N], FP8, kind="ExternalInput").ap()
        wq_d = nc.dram_tensor("wq", [D, DG], FP8, kind="ExternalInput").ap()
        wk_d = nc.dram_tensor("wk", [D, DG], FP8, kind="ExternalInput").ap()
    else:
        wq_d = nc.dram_tensor("wq", [D, DG], BF16, kind="ExternalInput").ap()
        wk_d = nc.dram_tensor("wk", [D, DG], BF16, kind="ExternalInput").ap()
    wv_d = nc.dram_tensor("wv", [D, DG], BF16, kind="ExternalInput").ap()
    wo_d = nc.dram_tensor("wo", [DG, D], BF16, kind="ExternalInput").ap()
    # partial outputs leave in bf16; the host sums the 4 groups in f32
    out_d = nc.dram_tensor("out", [N, D], BF16, kind="ExternalOutput").ap()

    with tile.TileContext(nc) as tc, ExitStack() as ctx:
        persist = ctx.enter_context(tc.tile_pool(name="persist", bufs=1))
        ptpool = ctx.enter_context(tc.tile_pool(name="ptpool", bufs=PT_BUFS))
        smpool = ctx.enter_context(tc.tile_pool(name="smpool", bufs=4))
        cspool = ctx.enter_context(tc.tile_pool(name="cspool", bufs=4))
        opool = ctx.enter_context(tc.tile_pool(name="opool", bufs=3))
        # PSUM (8 banks): "ps" [128,512] x2 (proj drains, ctx transposes,
        # out-proj) = 2; "ps_s" [128,2,512] x2 (S chunks) = 4; "ctx"
        # [128,2,65] x2 (PV accumulation) = 2.
        mmpsum = ctx.enter_context(tc.tile_pool(name="mmpsum", bufs=2, space="PSUM"))
        spsum = ctx.enter_context(tc.tile_pool(name="spsum", bufs=2, space="PSUM"))
        cpsum = ctx.enter_context(tc.tile_pool(name="cpsum", bufs=2, space="PSUM"))

        # ---- persistent tensors ----
        xT = persist.tile([128, NF, N], BF16, name="xT")          # 32 KB/p
        qt = persist.tile([128, 2, N], BF16, name="qt")           # 8 KB/p
        kt = persist.tile([128, 2, N], BF16, name="kt")           # 8 KB/p
        vt = persist.tile([128, NT, HG, HD + 1], BF16, name="vt")  # ~8 KB/p
        ctxT = persist.tile([128, 2, N], BF16, name="ctxT")       # 8 KB/p
        if FP8_QK:
            xT8 = persist.tile([128, NF, N], FP8, name="xT8")     # 16 KB/p
            # [f-pair, k-tile, head pair, d]: lhsT for one DoubleRow matmul
            wq_bf = persist.tile([128, NF // 2, 2, 2, 128], FP8, name="wq_f8")
            wk_bf = persist.tile([128, NF // 2, 2, 2, 128], FP8, name="wk_f8")
        else:
            wq_bf = persist.tile([128, NF, DG], BF16, name="wq_bf")  # 2 KB/p
            wk_bf = persist.tile([128, NF, DG], BF16, name="wk_bf")
        wv_bf = persist.tile([128, NF, DG], BF16, name="wv_bf")
        wo_bf = persist.tile([128, 2, D], BF16, name="wo_bf")     # 4 KB/p
        ident = persist.tile([128, 128], BF16, name="ident")
        cmask = persist.tile([128, 128], BF16, name="cmask")

        # identity (for ctx transposes) and causal mask, built once
        nc.gpsimd.memset(ident[:, :], 0.0)
        nc.gpsimd.affine_select(
            out=ident[:, :], in_=ident[:, :],
            compare_op=mybir.AluOpType.not_equal, fill=1.0,
            base=0, pattern=[[1, 128]], channel_multiplier=-1,
        )
        # cmask[k, q] = 1 where q >= k (keep at/below diagonal of P^T)
        nc.gpsimd.memset(cmask[:, :], 1.0)
        nc.gpsimd.affine_select(
            out=cmask[:, :], in_=cmask[:, :],
            compare_op=mybir.AluOpType.is_ge, fill=0.0,
            base=0, pattern=[[1, 128]], channel_multiplier=-1,
        )
        nc.gpsimd.memset(vt[:, :, :, HD], 1.0)  # softmax-sum ones columns

        def emit_weights_qkv():
            if FP8_QK:
                for w_dram, w8 in ((wq_d, wq_bf), (wk_d, wk_bf)):
                    nc.sync.dma_start(
                        out=w8[:, :, :, :, :],
                        in_=w_dram.rearrange("(a b p) (g e) -> p a b g e",
                                             a=NF // 2, b=2, p=128, g=2))
                nc.sync.dma_start(out=wv_bf[:, :, :],
                                  in_=wv_d.rearrange("(c p) d -> p c d", p=128))
                return
            for w_dram, w_bf in ((wq_d, wq_bf), (wk_d, wk_bf), (wv_d, wv_bf)):
                nc.sync.dma_start(out=w_bf[:, :, :],
                                  in_=w_dram.rearrange("(c p) d -> p c d", p=128))

        def emit_weights_o():
            nc.sync.dma_start(out=wo_bf[:, :, :],
                              in_=wo_d.rearrange("(c p) d -> p c d", p=128))

        def emit_xt_dma(ib):
            nc.sync.dma_start(
                out=xT[:, :, 512 * ib:512 * (ib + 1)],
                in_=x_d.rearrange("(c p) t -> p c t", p=128)[
                    :, :, 512 * ib:512 * (ib + 1)],
            )
            if FP8_QK:
                nc.sync.dma_start(
                    out=xT8[:, :, 512 * ib:512 * (ib + 1)],
                    in_=x8_d.rearrange("(c p) t -> p c t", p=128)[
                        :, :, 512 * ib:512 * (ib + 1)],
                )

        def proj_units(ib):
            """Projection of one 512-token block as filler units."""
            tb = ib
            units = []
            if FP8_QK:
                for w8, dst in ((wq_bf, qt), (wk_bf, kt)):
                    for p in range(2):
                        for tc2 in range(2):
                            def u(w8=w8, dst=dst, p=p, tc2=tc2):
                                tsl = slice(512 * tb + 256 * tc2,
                                            512 * tb + 256 * (tc2 + 1))
                                ps = mmpsum.tile([128, 512], FP32, name="ps",
                                                 tag="ps")
                                for fcp in range(NF // 2):
                                    nc.tensor.matmul(
                                        ps[:, 0:256],
                                        lhsT=w8[:, fcp, :, p, :],
                                        rhs=xT8[:, 2 * fcp:2 * fcp + 2, tsl],
                                        start=(fcp == 0),
                                        stop=(fcp == NF // 2 - 1),
                                        perf_mode=DR,
                                    )
                                nc.vector.tensor_copy(dst[:, p, tsl],
                                                      ps[:, 0:256])
                            units.append(u)
            else:
                for w_bf, dst in ((wq_bf, qt), (wk_bf, kt)):
                    for dh in range(2):
                        def u(w_bf=w_bf, dst=dst, dh=dh):
                            ps = mmpsum.tile([128, 512], FP32, name="ps",
                                             tag="ps")
                            for fc in range(NF):
                                nc.tensor.matmul(
                                    ps[:, :],
                                    lhsT=w_bf[:, fc, 128 * dh:128 * (dh + 1)],
                                    rhs=xT[:, fc, 512 * tb:512 * (tb + 1)],
                                    start=(fc == 0), stop=(fc == NF - 1),
                                )
                            nc.vector.tensor_copy(
                                dst[:, dh, 512 * tb:512 * (tb + 1)], ps[:, :])
                        units.append(u)
            for tcc in range(4 * ib, 4 * ib + 4):
                def u(tcc=tcc):
                    ps = mmpsum.tile([128, 512], FP32, name="ps", tag="ps")
                    for fc in range(NF):
                        nc.tensor.matmul(
                            ps[:, 0:DG],
                            lhsT=xT[:, fc, 128 * tcc:128 * (tcc + 1)],
                            rhs=wv_bf[:, fc, :],
                            start=(fc == 0), stop=(fc == NF - 1),
                        )
                    nc.vector.tensor_copy(
                        vt[:, tcc, :, 0:HD],
                        ps[:, 0:DG].rearrange("p (h e) -> p h e", h=HG))
                units.append(u)
            return units

        def emit_chunk(qb, p, kc, pts):
            """One S+exp(+mask) chunk for query block qb, head pair p."""
            ksl = slice(128 * kc, 128 * (kc + 1))
            m = max(0, kc - 4 * qb)
            q0 = 128 * m
            ps_s = spsum.tile([128, 2, 512], FP32, name="ps_s", tag="ps_s")
            for i in range(2):
                lo = 64 * i
                nc.tensor.matmul(
                    ps_s[:, i, q0:512],
                    lhsT=kt[lo:lo + 64, p, ksl],
                    rhs=qt[lo:lo + 64, p, 512 * qb + q0:512 * (qb + 1)],
                    start=True, stop=True,
                )
            pt = ptpool.tile([128, 2, 512], BF16, name="pt", tag="pt")
            esc = SCALE / (W8SCALE * W8SCALE) if FP8_QK else SCALE
            nc.scalar.activation(pt[:, :, q0:512], ps_s[:, :, q0:512], EXP,
                                 scale=esc)
            if kc >= 4 * qb:    # diagonal chunk: mask the [q0, q0+128) block
                for i in range(2):
                    nc.vector.tensor_mul(pt[:, i, q0:q0 + 128],
                                         pt[:, i, q0:q0 + 128], cmask[:, :])
            pts.append(pt)

        def pv_unit(qb, p, u, pts, trs):
            """PV + normalize for query sub-chunk u of (qb, p)."""
            nkk = 4 * qb + u + 1
            ctx_ps = cpsum.tile([128, 2, HD + 1], FP32, name="ctx_ps",
                                tag="ctx")
            for i in range(2):
                for kc in range(nkk):
                    nc.tensor.matmul(
                        ctx_ps[:, i, :],
                        lhsT=pts[kc][:, i, 128 * u:128 * (u + 1)],
                        rhs=vt[:, kc, 2 * p + i, :],
                        start=(kc == 0), stop=(kc == nkk - 1),
                    )
            rec = smpool.tile([128, 2], FP32, name="rec")
            with nc.allow_low_precision(reason="softmax reciprocal"):
                nc.vector.reciprocal(rec[:, :], ctx_ps[:, :, HD])
            ctx_sb = cspool.tile([128, 2, HD], BF16, name="ctx_sb")
            for i in range(2):
                nc.vector.tensor_scalar_mul(
                    ctx_sb[:, i, :], ctx_ps[:, i, 0:HD], rec[:, i:i + 1])
            trs[u] = ctx_sb

        def t_unit(qb, p, trs):
            """Transpose the 4 normalized ctx tiles into ctxT."""
            qsl = slice(512 * qb, 512 * (qb + 1))
            ps_tr = mmpsum.tile([128, 512], BF16, name="ps_tr", tag="ps")
            for u in range(4):
                nc.tensor.transpose(
                    ps_tr[:, 128 * u:128 * (u + 1)],
                    trs[u][:, :, :].rearrange("p i e -> p (i e)"),
                    ident[:, :],
                )
            nc.vector.tensor_copy(ctxT[:, p, qsl], ps_tr[:, :])

        def outproj_units(qb):
            units = []
            for tb in range(4 * qb, 4 * qb + 4):
                tsl = slice(128 * tb, 128 * (tb + 1))
                for nh in range(2):
                    # alternate drains between Pool and DVE so neither
                    # paces the 2-slot psum rotation; the trailing qb=3
                    # block goes all-DVE (faster, and DVE is idle then)
                    on_pool = GPSIMD_DRAIN and qb != NQ - 1 and nh == 0

                    def u(tsl=tsl, nh=nh, on_pool=on_pool):
                        ps_o = mmpsum.tile([128, 512], FP32, name="ps",
                                           tag="ps")
                        for hc in range(2):
                            nc.tensor.matmul(
                                ps_o[:, :],
                                lhsT=ctxT[:, hc, tsl],
                                rhs=wo_bf[:, hc, 512 * nh:512 * (nh + 1)],
                                start=(hc == 0), stop=(hc == 1),
                            )
                        o_sb = opool.tile([128, 512], BF16, name="o_sb")
                        if on_pool:
                            nc.gpsimd.tensor_copy(o_sb[:, :], ps_o[:, :])
                        else:
                            nc.vector.tensor_copy(o_sb[:, :], ps_o[:, :])
                        nc.sync.dma_start(
                            out=out_d[tsl, 512 * nh:512 * (nh + 1)],
                            in_=o_sb[:, :])
                        if FENCE and tb == NT - 1 and nh == 1:
                            # write-after-read fence: next body's x DMA
                            # (a writer of xT block 0) must wait for this
                            # tiny write, which depends on the last output
                            nc.vector.tensor_copy(xT[0:1, 0, 0:1],
                                                  o_sb[0:1, 0:1])
                    units.append(u)
            return units

        def emit_body():
            # x block 0 first: the first projection unit only needs x0 + wq
            emit_xt_dma(0)
            emit_weights_qkv()
            emit_xt_dma(1)
            emit_weights_o()
            # proj(0) must be complete before the chunk stream starts
            for u in proj_units(0):
                u()

            # ---- software-pipelined main stream ----
            # chunk stream positions: (qb, p, kc) in order
            stream = [(qb, p, kc)
                      for qb in range(NQ) for p in range(2)
                      for kc in range(4 * (qb + 1))]
            group_start = {}
            pos = 0
            for qb in range(NQ):
                for p in range(2):
                    group_start[(qb, p)] = pos
                    pos += 4 * (qb + 1)

            # filler schedule: (ready_pos, order, emit_fn)
            pend = []

            def add(ready, fn):
                pend.append([ready, len(pend), fn])

            # projections of blocks 1..3, spread so block ib drains before
            # its attention group starts
            for ib, (lo, hi) in ((1, (0, 7)), (2, (8, 22)), (3, (24, 46))):
                us = proj_units(ib)
                for j, u in enumerate(us):
                    add(lo + j * max(1, (hi - lo) // len(us)), u)

            # xT DMAs for blocks 2,3 a little ahead of their proj fillers
            dma_at = {4: 2, 16: 3}

            pts_map = {}
            trs_map = {}
            t_ready = {}
            for qb in range(NQ):
                for p in range(2):
                    g = group_start[(qb, p)]
                    pts_map[(qb, p)] = []
                    trs_map[(qb, p)] = {}
                    last = 0
                    for u in range(4):
                        ready = g + 4 * qb + u + 2
                        add(ready, (lambda qb=qb, p=p, u=u:
                                    pv_unit(qb, p, u, pts_map[(qb, p)],
                                            trs_map[(qb, p)])))
                        last = ready
                    add(last + 2, (lambda qb=qb, p=p:
                                   t_unit(qb, p, trs_map[(qb, p)])))
                    t_ready[(qb, p)] = last + 2
            # all output projections go into the filler-starved qb=3 chunk
            # region (positions 48..79, where proj fillers have run dry and
            # ScalarE paces the stream); qb=3's own outproj trails the stream
            j = 0
            for qb in range(NQ - 1):
                for u in outproj_units(qb):
                    add(max(t_ready[(qb, 1)] + 1, 48 + (5 * j) // 4), u)
                    j += 1
            for j, u in enumerate(outproj_units(NQ - 1)):
                add(t_ready[(NQ - 1, 1)] + 1 + j, u)

            pend.sort(key=lambda e: (e[0], e[1]))
            pi = 0
            for posi, (qb, p, kc) in enumerate(stream):
                if posi in dma_at:
                    emit_xt_dma(dma_at[posi])
                while pi < len(pend) and pend[pi][0] <= posi:
                    pend[pi][2]()
                    pi += 1
                emit_chunk(qb, p, kc, pts_map[(qb, p)])
            while pi < len(pend):
                pend[pi][2]()
                pi += 1

        for _rep in range(repeat):
            emit_body()

    nc.compile()
    return nc


def _get_nc(repeat=1):
    key = ("nc", repeat)
    if key not in _CACHE:
        _CACHE[key] = _build_nc(repeat)
    return _CACHE[key]


def _make_in_maps(x, Wq, Wk, Wv, Wo):
    import ml_dtypes
    bf = ml_dtypes.bfloat16
    f8 = ml_dtypes.float8_e4m3
    in_maps = []
    for c in range(NCORES):
        b, g = divmod(c, 4)
        cs = slice(DG * g, DG * (g + 1))
        xb = np.ascontiguousarray(x[b].T).astype(bf)
        m = {
            "x": xb,
            "wv": np.ascontiguousarray(Wv[:, cs]).astype(bf),
            "wo": np.ascontiguousarray(Wo[cs, :]).astype(bf),
        }
        if FP8_QK:
            m["x8"] = xb.astype(f8)
            m["wq"] = (np.ascontiguousarray(Wq[:, cs]) * W8SCALE).astype(f8)
            m["wk"] = (np.ascontiguousarray(Wk[:, cs]) * W8SCALE).astype(f8)
        else:
            m["wq"] = np.ascontiguousarray(Wq[:, cs]).astype(bf)
            m["wk"] = np.ascontiguousarray(Wk[:, cs]).astype(bf)
        in_maps.append(m)
    return in_maps


def _gather(results, bo):
    out = np.empty((2, N, D), dtype=np.float32)
    for b in range(2):
        acc = results[4 * b]["out"].astype(np.float32)
        for g in range(1, 4):
            acc = acc + results[4 * b + g]["out"]
        out[b] = acc + bo[None, :].astype(np.float32)
    return out


def run_spmd(x, Wq, Wk, Wv, Wo, bo, **spmd_kwargs):
    """Run the 8-core kernel; returns (full_output, BassKernelResults)."""
    from concourse.bass_utils import run_bass_kernel_spmd

    nc = _get_nc()
    in_maps = _make_in_maps(
        np.asarray(x), np.asarray(Wq), np.asarray(Wk), np.asarray(Wv),
        np.asarray(Wo))
    res = run_bass_kernel_spmd(nc, in_maps, core_ids=list(range(NCORES)),
                               **spmd_kwargs)
    return _gather(res.results, np.asarray(bo)), res


def kernel(x, Wq, Wk, Wv, Wo, bo):
    out, _ = run_spmd(x, Wq, Wk, Wv, Wo, bo)
    return out


# revision 41
# speedup vs baseline: 2.7570x; 2.7570x over previous
"""Causal multi-head attention on 8 Trainium2 NeuronCores.

Full module: x:(2,2048,1024) f32, 16 heads, head_dim 64, causal softmax
(scaled by 1/sqrt(1024)), out = attn(x) @ Wo + bo.

Sharding: core c handles batch b = c // 4 and head group g = c % 4
(4 heads = 2 head pairs, i.e. 256 columns of Wq/Wk/Wv and 256 rows of
Wo). Every core runs the same program (SPMD); the host sums the 4
per-group partial output projections per batch and adds the bias.

Per-core layout (all matmuls contract over the SBUF partition dim):
  xT   [f=1024, t=2048] bf16, transposed on the host, straight DMA in
  QT/KT[d, t] per head pair: partitions = d within pair (head A 0-63,
       head B 64-127), built from Wq/Wk as lhsT against xT
  V    [t, d] token-major [128, 16 chunks, 4 heads, 64+1] with a fused
       ones column per head (softmax sums fall out of the PV matmul)
  S^T  [k=128, q<=512] per chunk = KT-slice lhsT x QT rhs, two heads on
       disjoint 64-partition halves, diagonal chunks narrowed to the
       causal column range
  P^T  = exp(S^T/32) on ScalarE; diagonal 128-block masked by a bf16
       lower-triangle multiply on DVE
  ctx  [q=128, 65] per query sub-chunk = pt-slice lhsT x V rhs,
       accumulated over k chunks in PSUM; col 64 = softmax sums
  norm: reciprocal(sums) -> per-partition tensor_scalar multiply (DVE)
  ctxT via PE transpose of the normalized [q, d] tile
  out  [t, 1024] = ctxT lhsT x Wo rhs (partial over heads; host sums)

Emission is software-pipelined: the 80 S/exp chunk units pace the body
and every other PE-side unit (projections of later token blocks, PV,
transposes, output projection) is interleaved between chunks with a
small lag so no engine queue head-blocks on ScalarE.
"""

import os

import numpy as np

N = 2048        # tokens per batch
D = 1024        # model dim
HG = 4          # heads per core
HD = 64         # head dim
DG = HG * HD    # 256, feature columns per core
SCALE = 1.0 / 32.0  # 1/sqrt(D); module scales by sqrt(d_out), not head_dim
NCORES = 8

PT_BUFS = int(os.environ.get("PT_BUFS", "22"))
# gpsimd cannot read PSUM (BIR verifier) — drains stay on DVE
GPSIMD_DRAIN = os.environ.get("GPSIMD_DRAIN", "0") != "0"
# fp8 (e4m3) Q/K projections via DoubleRow matmuls (2x PE throughput).
# Wq/Wk are pre-scaled by W8SCALE into fp8 range; the exp() scale divides
# the resulting alpha^2 factor back out of the scores.
FP8_QK = os.environ.get("FP8_QK", "1") != "0"
W8SCALE = 64.0
# FENCE=1 serializes repetition bodies (for latency measurement): the next
# body's first x DMA is made to wait on the previous body's last output.
FENCE = os.environ.get("FENCE", "0") != "0"

_CACHE = {}


def _build_nc(repeat=1):
    from contextlib import ExitStack

    import concourse.mybir as mybir
    import concourse.tile as tile
    from concourse import bacc

    FP32 = mybir.dt.float32
    BF16 = mybir.dt.bfloat16
    FP8 = mybir.dt.float8e4
    EXP = mybir.ActivationFunctionType.Exp
    DR = mybir.MatmulPerfMode.DoubleRow

    NT = N // 128   # 16 token chunks
    NF = D // 128   # 8 feature chunks
    NQ = N // 512   # 4 query blocks

    nc = bacc.Bacc("TRN2", target_bir_lowering=False, debug=False)

    x_d = nc.dram_tensor("x", [D, N], BF16, kind="ExternalInput").ap()
    if FP8_QK:
        x8_d = nc.dram_tensor("x8", [D, N], FP8, kind="ExternalInput").ap()
        wq_d = nc.dram_tensor("wq", [D, DG], FP8, kind="ExternalInput").ap()
        wk_d = nc.dram_tensor("wk", [D, DG], FP8, kind="ExternalInput").ap()
    else:
        wq_d = nc.dram_tensor("wq", [D, DG], BF16, kind="ExternalInput").ap()
        wk_d = nc.dram_tensor("wk", [D, DG], BF16, kind="ExternalInput").ap()
    wv_d = nc.dram_tensor("wv", [D, DG], BF16, kind="ExternalInput").ap()
    wo_d = nc.dram_tensor("wo", [DG, D], BF16, kind="ExternalInput").ap()
    # partial outputs leave in bf16; the host sums the 4 groups in f32
    out_d = nc.dram_tensor("out", [N, D], BF16, kind="ExternalOutput").ap()

    with tile.TileContext(nc) as tc, ExitStack() as ctx:
        persist = ctx.enter_context(tc.tile_pool(name="persist", bufs=1))
        ptpool = ctx.enter_context(tc.tile_pool(name="ptpool", bufs=PT_BUFS))
        smpool = ctx.enter_context(tc.tile_pool(name="smpool", bufs=4))
        cspool = ctx.enter_context(tc.tile_pool(name="cspool", bufs=4))
        opool = ctx.enter_context(tc.tile_pool(name="opool", bufs=6))
        # PSUM (8 banks): "ps" [128,512] x2 (proj drains, ctx transposes,
        # out-proj) = 2; "ps_s" [128,2,512] x2 (S chunks) = 4; "ctx"
        # [128,2,65] x2 (PV accumulation) = 2.
        mmpsum = ctx.enter_context(tc.tile_pool(name="mmpsum", bufs=2, space="PSUM"))
        spsum = ctx.enter_context(tc.tile_pool(name="spsum", bufs=2, space="PSUM"))
        cpsum = ctx.enter_context(tc.tile_pool(name="cpsum", bufs=2, space="PSUM"))

        # ---- persistent tensors ----
        xT = persist.tile([128, NF, N], BF16, name="xT")          # 32 KB/p
        qt = persist.tile([128, 2, N], BF16, name="qt")           # 8 KB/p
        kt = persist.tile([128, 2, N], BF16, name="kt")           # 8 KB/p
        vt = persist.tile([128, NT, HG, HD + 1], BF16, name="vt")  # ~8 KB/p
        ctxT = persist.tile([128, 2, N], BF16, name="ctxT")       # 8 KB/p
        if FP8_QK:
            xT8 = persist.tile([128, NF, N], FP8, name="xT8")     # 16 KB/p
            # [f-pair, k-tile, head pair, d]: lhsT for one DoubleRow matmul
            wq_bf = persist.tile([128, NF // 2, 2, 2, 128], FP8, name="wq_f8")
            wk_bf = persist.tile([128, NF // 2, 2, 2, 128], FP8, name="wk_f8")
        else:
            wq_bf = persist.tile([128, NF, DG], BF16, name="wq_bf")  # 2 KB/p
            wk_bf = persist.tile([128, NF, DG], BF16, name="wk_bf")
        wv_bf = persist.tile([128, NF, DG], BF16, name="wv_bf")
        wo_bf = persist.tile([128, 2, D], BF16, name="wo_bf")     # 4 KB/p
        ident = persist.tile([128, 128], BF16, name="ident")
        cmask = persist.tile([128, 128], BF16, name="cmask")

        # identity (for ctx transposes) and causal mask, built once
        nc.gpsimd.memset(ident[:, :], 0.0)
        nc.gpsimd.affine_select(
            out=ident[:, :], in_=ident[:, :],
            compare_op=mybir.AluOpType.not_equal, fill=1.0,
            base=0, pattern=[[1, 128]], channel_multiplier=-1,
        )
        # cmask[k, q] = 1 where q >= k (keep at/below diagonal of P^T)
        nc.gpsimd.memset(cmask[:, :], 1.0)
        nc.gpsimd.affine_select(
            out=cmask[:, :], in_=cmask[:, :],
            compare_op=mybir.AluOpType.is_ge, fill=0.0,
            base=0, pattern=[[1, 128]], channel_multiplier=-1,
        )
        nc.gpsimd.memset(vt[:, :, :, HD], 1.0)  # softmax-sum ones columns

        def emit_weights_qk():
            if FP8_QK:
                for w_dram, w8 in ((wq_d, wq_bf), (wk_d, wk_bf)):
                    nc.sync.dma_start(
                        out=w8[:, :, :, :, :],
                        in_=w_dram.rearrange("(a b p) (g e) -> p a b g e",
                                             a=NF // 2, b=2, p=128, g=2))
                return
            for w_dram, w_bf in ((wq_d, wq_bf), (wk_d, wk_bf)):
                nc.sync.dma_start(out=w_bf[:, :, :],
                                  in_=w_dram.rearrange("(c p) d -> p c d", p=128))

        def emit_weights_v():
            nc.sync.dma_start(out=wv_bf[:, :, :],
                              in_=wv_d.rearrange("(c p) d -> p c d", p=128))

        def emit_weights_o():
            nc.sync.dma_start(out=wo_bf[:, :, :],
                              in_=wo_d.rearrange("(c p) d -> p c d", p=128))

        def emit_xt_dma(ib, only=None):
            # fp8 copy first: it is half the bytes and unblocks Q/K proj
            if FP8_QK and only != "bf16":
                nc.sync.dma_start(
                    out=xT8[:, :, 512 * ib:512 * (ib + 1)],
                    in_=x8_d.rearrange("(c p) t -> p c t", p=128)[
                        :, :, 512 * ib:512 * (ib + 1)],
                )
            if only != "fp8":
                nc.sync.dma_start(
                    out=xT[:, :, 512 * ib:512 * (ib + 1)],
                    in_=x_d.rearrange("(c p) t -> p c t", p=128)[
                        :, :, 512 * ib:512 * (ib + 1)],
                )

        def proj_units(ib):
            """Projection of one 512-token block as filler units."""
            tb = ib
            units = []
            if FP8_QK:
                for w8, dst in ((wq_bf, qt), (wk_bf, kt)):
                    for p in range(2):
                        for tc2 in range(2):
                            def u(w8=w8, dst=dst, p=p, tc2=tc2):
                                tsl = slice(512 * tb + 256 * tc2,
                                            512 * tb + 256 * (tc2 + 1))
                                ps = mmpsum.tile([128, 512], FP32, name="ps",
                                                 tag="ps")
                                for fcp in range(NF // 2):
                                    nc.tensor.matmul(
                                        ps[:, 0:256],
                                        lhsT=w8[:, fcp, :, p, :],
                                        rhs=xT8[:, 2 * fcp:2 * fcp + 2, tsl],
                                        start=(fcp == 0),
                                        stop=(fcp == NF // 2 - 1),
                                        perf_mode=DR,
                                    )
                                nc.vector.tensor_copy(dst[:, p, tsl],
                                                      ps[:, 0:256])
                            units.append(u)
            else:
                for w_bf, dst in ((wq_bf, qt), (wk_bf, kt)):
                    for dh in range(2):
                        def u(w_bf=w_bf, dst=dst, dh=dh):
                            ps = mmpsum.tile([128, 512], FP32, name="ps",
                                             tag="ps")
                            for fc in range(NF):
                                nc.tensor.matmul(
                                    ps[:, :],
                                    lhsT=w_bf[:, fc, 128 * dh:128 * (dh + 1)],
                                    rhs=xT[:, fc, 512 * tb:512 * (tb + 1)],
                                    start=(fc == 0), stop=(fc == NF - 1),
                                )
                            nc.vector.tensor_copy(
                                dst[:, dh, 512 * tb:512 * (tb + 1)], ps[:, :])
                        units.append(u)
            for tcc in range(4 * ib, 4 * ib + 4):
                def u(tcc=tcc):
                    ps = mmpsum.tile([128, 512], FP32, name="ps", tag="ps")
                    for fc in range(NF):
                        nc.tensor.matmul(
                            ps[:, 0:DG],
                            lhsT=xT[:, fc, 128 * tcc:128 * (tcc + 1)],
                            rhs=wv_bf[:, fc, :],
                            start=(fc == 0), stop=(fc == NF - 1),
                        )
                    nc.vector.tensor_copy(
                        vt[:, tcc, :, 0:HD],
                        ps[:, 0:DG].rearrange("p (h e) -> p h e", h=HG))
                units.append(u)
            return units

        def emit_chunk(qb, p, kc, pts):
            """One S+exp(+mask) chunk for query block qb, head pair p."""
            ksl = slice(128 * kc, 128 * (kc + 1))
            m = max(0, kc - 4 * qb)
            q0 = 128 * m
            ps_s = spsum.tile([128, 2, 512], FP32, name="ps_s", tag="ps_s")
            for i in range(2):
                lo = 64 * i
                nc.tensor.matmul(
                    ps_s[:, i, q0:512],
                    lhsT=kt[lo:lo + 64, p, ksl],
                    rhs=qt[lo:lo + 64, p, 512 * qb + q0:512 * (qb + 1)],
                    start=True, stop=True,
                )
            pt = ptpool.tile([128, 2, 512], BF16, name="pt", tag="pt")
            esc = SCALE / (W8SCALE * W8SCALE) if FP8_QK else SCALE
            nc.scalar.activation(pt[:, :, q0:512], ps_s[:, :, q0:512], EXP,
                                 scale=esc)
            if kc >= 4 * qb:    # diagonal chunk: mask the [q0, q0+128) block
                for i in range(2):
                    nc.vector.tensor_mul(pt[:, i, q0:q0 + 128],
                                         pt[:, i, q0:q0 + 128], cmask[:, :])
            pts.append(pt)

        def pv_unit(qb, p, u, pts, trs):
            """PV + normalize for query sub-chunk u of (qb, p)."""
            nkk = 4 * qb + u + 1
            ctx_ps = cpsum.tile([128, 2, HD + 1], FP32, name="ctx_ps",
                                tag="ctx")
            for i in range(2):
                for kc in range(nkk):
                    nc.tensor.matmul(
                        ctx_ps[:, i, :],
                        lhsT=pts[kc][:, i, 128 * u:128 * (u + 1)],
                        rhs=vt[:, kc, 2 * p + i, :],
                        start=(kc == 0), stop=(kc == nkk - 1),
                    )
            rec = smpool.tile([128, 2], FP32, name="rec")
            with nc.allow_low_precision(reason="softmax reciprocal"):
                nc.vector.reciprocal(rec[:, :], ctx_ps[:, :, HD])
            ctx_sb = cspool.tile([128, 2, HD], BF16, name="ctx_sb")
            for i in range(2):
                nc.vector.tensor_scalar_mul(
                    ctx_sb[:, i, :], ctx_ps[:, i, 0:HD], rec[:, i:i + 1])
            trs[u] = ctx_sb

        def t_unit(qb, p, u, trs):
            """Transpose one normalized ctx tile into ctxT."""
            tsl = slice(512 * qb + 128 * u, 512 * qb + 128 * (u + 1))
            ps_tr = mmpsum.tile([128, 512], BF16, name="ps_tr", tag="ps")
            nc.tensor.transpose(
                ps_tr[:, 0:128],
                trs[u][:, :, :].rearrange("p i e -> p (i e)"),
                ident[:, :],
            )
            nc.vector.tensor_copy(ctxT[:, p, tsl], ps_tr[:, 0:128])

        def outproj_units(qb, tbs=None):
            units = []
            for tb in (tbs if tbs is not None else range(4 * qb, 4 * qb + 4)):
                tsl = slice(128 * tb, 128 * (tb + 1))
                for nh in range(2):
                    # alternate drains between Pool and DVE so neither
                    # paces the 2-slot psum rotation; the trailing qb=3
                    # block goes all-DVE (faster, and DVE is idle then)
                    on_pool = GPSIMD_DRAIN and qb != NQ - 1 and nh == 0

                    def u(tb=tb, tsl=tsl, nh=nh, on_pool=on_pool):
                        ps_o = mmpsum.tile([128, 512], FP32, name="ps",
                                           tag="ps")
                        for hc in range(2):
                            nc.tensor.matmul(
                                ps_o[:, :],
                                lhsT=ctxT[:, hc, tsl],
                                rhs=wo_bf[:, hc, 512 * nh:512 * (nh + 1)],
                                start=(hc == 0), stop=(hc == 1),
                            )
                        o_sb = opool.tile([128, 512], BF16, name="o_sb")
                        if on_pool:
                            nc.gpsimd.tensor_copy(o_sb[:, :], ps_o[:, :])
                        else:
                            nc.vector.tensor_copy(o_sb[:, :], ps_o[:, :])
                        nc.sync.dma_start(
                            out=out_d[tsl, 512 * nh:512 * (nh + 1)],
                            in_=o_sb[:, :])
                        if FENCE and tb == NT - 1 and nh == 1:
                            # write-after-read fence: next body's x DMA
                            # (a writer of xT block 0) must wait for this
                            # tiny write, which depends on the last output
                            nc.vector.tensor_copy(xT[0:1, 0, 0:1],
                                                  o_sb[0:1, 0:1])
                    units.append(u)
            return units

        def emit_body():
            # DMA order mirrors the first consumers: x(8) block 0 and the
            # q/k weights unblock the first projection units
            emit_xt_dma(0, only="fp8")
            emit_weights_qk()
            emit_xt_dma(0, only="bf16")
            emit_weights_v()
            emit_xt_dma(1)
            emit_weights_o()
            # proj(0) must be complete before the chunk stream starts
            for u in proj_units(0):
                u()

            # ---- software-pipelined main stream ----
            # chunk stream positions: (qb, p, kc) in order
            stream = [(qb, p, kc)
                      for qb in range(NQ) for p in range(2)
                      for kc in range(4 * (qb + 1))]
            group_start = {}
            pos = 0
            for qb in range(NQ):
                for p in range(2):
                    group_start[(qb, p)] = pos
                    pos += 4 * (qb + 1)

            # filler schedule: (ready_pos, order, emit_fn)
            pend = []

            def add(ready, fn):
                pend.append([ready, len(pend), fn])

            # projections of blocks 1..3, spread so block ib drains before
            # its attention group starts
            for ib, (lo, hi) in ((1, (0, 7)), (2, (8, 22)), (3, (24, 46))):
                us = proj_units(ib)
                for j, u in enumerate(us):
                    add(lo + j * max(1, (hi - lo) // len(us)), u)

            # xT DMAs for blocks 2,3 a little ahead of their proj fillers
            dma_at = {4: 2, 16: 3}

            pts_map = {}
            trs_map = {}
            t1_ready = {}
            for qb in range(NQ):
                for p in range(2):
                    g = group_start[(qb, p)]
                    pts_map[(qb, p)] = []
                    trs_map[(qb, p)] = {}
                    for u in range(4):
                        ready = g + 4 * qb + u + 3
                        add(ready, (lambda qb=qb, p=p, u=u:
                                    pv_unit(qb, p, u, pts_map[(qb, p)],
                                            trs_map[(qb, p)])))
                        add(ready + 2, (lambda qb=qb, p=p, u=u:
                                        t_unit(qb, p, u, trs_map[(qb, p)])))
                        if p == 1:
                            t1_ready[(qb, u)] = ready + 2
            # output projections are the only movable PE filler once the
            # (cheap, fp8) projections drain; pack them into the qb=3
            # chunk region where ScalarE paces the stream and PE starves.
            # qb=3's own outproj trails the stream, pipelined per token
            # tile behind its transpose.
            op_pos = [50 + j for j in range(24)]
            j = 0
            for qb in range(NQ - 1):
                for tb in range(4 * qb, 4 * qb + 4):
                    for u in outproj_units(qb, tbs=[tb]):
                        add(max(t1_ready[(qb, tb - 4 * qb)] + 1, op_pos[j]), u)
                        j += 1
            for tb in range(4 * (NQ - 1), 4 * NQ):
                for k, u in enumerate(outproj_units(NQ - 1, tbs=[tb])):
                    add(t1_ready[(NQ - 1, tb - 4 * (NQ - 1))] + 1 + k, u)

            pend.sort(key=lambda e: (e[0], e[1]))
            pi = 0
            for posi, (qb, p, kc) in enumerate(stream):
                if posi in dma_at:
                    emit_xt_dma(dma_at[posi])
                while pi < len(pend) and pend[pi][0] <= posi:
                    pend[pi][2]()
                    pi += 1
                emit_chunk(qb, p, kc, pts_map[(qb, p)])
            while pi < len(pend):
                pend[pi][2]()
                pi += 1

        for _rep in range(repeat):
            emit_body()

    nc.compile()
    return nc


def _get_nc(repeat=1):
    key = ("nc", repeat)
    if key not in _CACHE:
        _CACHE[key] = _build_nc(repeat)
    return _CACHE[key]


def _make_in_maps(x, Wq, Wk, Wv, Wo):
    import ml_dtypes
    bf = ml_dtypes.bfloat16
    f8 = ml_dtypes.float8_e4m3
    in_maps = []
    for c in range(NCORES):
        b, g = divmod(c, 4)
        cs = slice(DG * g, DG * (g + 1))
        xb = np.ascontiguousarray(x[b].T).astype(bf)
        m = {
            "x": xb,
            "wv": np.ascontiguousarray(Wv[:, cs]).astype(bf),
            "wo": np.ascontiguousarray(Wo[cs, :]).astype(bf),
        }
        if FP8_QK:
            m["x8"] = xb.astype(f8)
            m["wq"] = (np.ascontiguousarray(Wq[:, cs]) * W8SCALE).astype(f8)
            m["wk"] = (np.ascontiguousarray(Wk[:, cs]) * W8SCALE).astype(f8)
        else:
            m["wq"] = np.ascontiguousarray(Wq[:, cs]).astype(bf)
            m["wk"] = np.ascontiguousarray(Wk[:, cs]).astype(bf)
        in_maps.append(m)
    return in_maps


def _gather(results, bo):
    out = np.empty((2, N, D), dtype=np.float32)
    for b in range(2):
        acc = results[4 * b]["out"].astype(np.float32)
        for g in range(1, 4):
            acc = acc + results[4 * b + g]["out"]
        out[b] = acc + bo[None, :].astype(np.float32)
    return out


def run_spmd(x, Wq, Wk, Wv, Wo, bo, **spmd_kwargs):
    """Run the 8-core kernel; returns (full_output, BassKernelResults)."""
    from concourse.bass_utils import run_bass_kernel_spmd

    nc = _get_nc()
    in_maps = _make_in_maps(
        np.asarray(x), np.asarray(Wq), np.asarray(Wk), np.asarray(Wv),
        np.asarray(Wo))
    res = run_bass_kernel_spmd(nc, in_maps, core_ids=list(range(NCORES)),
                               **spmd_kwargs)
    return _gather(res.results, np.asarray(bo)), res


def kernel(x, Wq, Wk, Wv, Wo, bo):
    out, _ = run_spmd(x, Wq, Wk, Wv, Wo, bo)
    return out
